# revision 1
# baseline (speedup 1.0000x reference)
"""Trainium2 Bass kernel for causal self-attention (B=4, T=2048, C=1024, H=16).

Sharding: 2 heads per core across 8 cores (tensor parallel on heads).
Per core:
  1. QKV projection for its 128 channels (2 heads), q/k/v kept transposed
     [ch, tok] in SBUF.  Projection matmuls run in float32r (TF32-like,
     full PE rate); all operands are DMA-fed fp32r so no rounding ops.
  2. Flash-style causal attention per (batch, head) in fp16 (values are
     O(1) so fp16's 10-bit mantissa is safe and runs at full PE rate).
     Scores are computed TRANSPOSED (S^T [s, t]) so the softmax denominator
     comes out of the same matmul that applies V: lhsT = [v_h | ones] makes
     PSUM rows 64:128 the row-sum Z (no max subtraction: |S*scale| < ~6).
  3. y^T blocks (fp16) are exchanged with an on-chip AllToAll so each core
     owns a 1024-token slice, then projects with the full Wp in fp16.
Host side: x is pre-transposed, per-core weight slices pre-sliced; output
slices are concatenated and bp added at the end.
"""

import numpy as np

import concourse.bass as bass
import concourse.mybir as mybir
import concourse.tile as tile
from concourse import bacc

F32 = mybir.dt.float32
F32R = mybir.dt.float32r
F16 = mybir.dt.float16
EXP = mybir.ActivationFunctionType.Exp

# problem shape (hardcoded per harness contract)
B, T, C, H = 4, 2048, 1024, 16
D = C // H              # 64
NCORES = 8
BT = B * T
TSL = BT // NCORES      # tokens per core after AllToAll
SCALE = 1.0 / np.sqrt(np.float32(D))


def build_program(b=B, t=T, c=C, ncores=NCORES, reps=1):
    """Build the SPMD single-core program. Requires c == 128 * ncores."""
    assert c == 128 * ncores, "2 heads of 64 dims per core"
    bt = b * t
    tsl = bt // ncores
    nk = c // 128            # contraction tiles for projections
    tch = t // 512           # 512-token chunks per batch
    sbk = t // 128           # 128-token s-blocks per batch
    n_out_ch = tsl // 512    # local out-proj token chunks

    nc = bacc.Bacc("TRN2", target_bir_lowering=False, num_devices=ncores)

    xT = nc.dram_tensor("xT", [c, bt], F16, kind="ExternalInput")
    wqT = nc.dram_tensor("wqT", [c, 128], F16, kind="ExternalInput")
    wkT = nc.dram_tensor("wkT", [c, 128], F16, kind="ExternalInput")
    wvT = nc.dram_tensor("wvT", [c, 128], F16, kind="ExternalInput")
    bq = nc.dram_tensor("bq", [128, 1], F32, kind="ExternalInput")
    bk = nc.dram_tensor("bk", [128, 1], F32, kind="ExternalInput")
    bv = nc.dram_tensor("bv", [128, 1], F32, kind="ExternalInput")
    wpT = nc.dram_tensor("wpT", [c, c], F16, kind="ExternalInput")
    ident = nc.dram_tensor("ident", [128, 128], F16, kind="ExternalInput")
    outT = nc.dram_tensor("outT", [c, tsl], F32, kind="ExternalOutput")

    with tile.TileContext(nc) as tc:
        with (
            tc.tile_pool(name="singles", bufs=1) as singles,
            tc.tile_pool(name="dram", bufs=1, space="DRAM") as dram,
            tc.tile_pool(name="xin", bufs=2) as xin,
            tc.tile_pool(name="qkv", bufs=2) as qkv,
            tc.tile_pool(name="vva", bufs=2) as vva,
            tc.tile_pool(name="ptile", bufs=3) as ptile,
            tc.tile_pool(name="ynorm", bufs=3) as ynorm,
            tc.tile_pool(name="wp", bufs=10) as wppool,
            tc.tile_pool(name="outsb", bufs=3) as outsb,
            tc.tile_pool(name="yg", bufs=2 * nk + 1) as ygpool,
            tc.tile_pool(name="ps_s", bufs=2, space="PSUM") as ps_s,
            tc.tile_pool(name="ps_y", bufs=1, space="PSUM") as ps_y,
            tc.tile_pool(name="ps_mm", bufs=2, space="PSUM") as ps_mm,
        ):
            a2a_in0 = dram.tile([ncores, 128, 512], F16)
            a2a_out0 = dram.tile([ncores, 128, 512], F16)
            a2a_in1 = dram.tile([ncores, 128, 512], F16)
            a2a_out1 = dram.tile([ncores, 128, 512], F16)
            a2a_ins = [a2a_in0, a2a_in1]
            a2a_outs = [a2a_out0, a2a_out1]

            # --- constants ---
            identity = singles.tile([128, 128], F16)
            nc.sync.dma_start(out=identity, in_=ident[:, :])
            cmask = singles.tile([128, 128], F32)
            nc.gpsimd.memset(cmask, 0.0)
            # keep (0) where t - s >= 0 else -1e10
            nc.gpsimd.affine_select(
                out=cmask, in_=cmask, compare_op=mybir.AluOpType.is_ge,
                fill=-1e10, base=0, channel_multiplier=-1, pattern=[[1, 128]],
            )
            w_tiles = {}
            bias_tiles = {}
            for nm, wt, bias in (("q", wqT, bq), ("k", wkT, bk), ("v", wvT, bv)):
                for ck in range(nk):
                    wtile = singles.tile([128, 128], F16, name=f"w{nm}_{ck}")
                    nc.sync.dma_start(out=wtile, in_=wt[128 * ck:128 * ck + 128, :])
                    w_tiles[nm, ck] = wtile
                btile = singles.tile([128, 1], F32, name=f"b{nm}")
                nc.sync.dma_start(out=btile, in_=bias[:, :])
                bias_tiles[nm] = btile

            # --- per batch: projection, v-prep, attention ---
            for _rep in range(reps):
              for bi in range(b):
                  t0 = bi * t
                  qT = qkv.tile([128, t], F16, tag="qT")
                  kT = qkv.tile([128, t], F16, tag="kT")
                  vT = qkv.tile([128, t], F16, tag="vT")
                  for j in range(tch):
                      xt = xin.tile([128, nk, 512], F16, tag="xt")
                      nc.sync.dma_start(
                          out=xt,
                          in_=xT.rearrange("(ck p) g -> p ck g", p=128)[
                              :, :, t0 + 512 * j:t0 + 512 * j + 512],
                      )
                      for nm, dst in (("q", qT), ("k", kT), ("v", vT)):
                          ps = ps_mm.tile([128, 512], F32, tag="mm")
                          for ck in range(nk):
                              nc.tensor.matmul(
                                  ps, w_tiles[nm, ck], xt[:, ck, :],
                                  start=(ck == 0), stop=(ck == nk - 1),
                              )
                          nc.vector.tensor_scalar_add(
                              dst[:, 512 * j:512 * j + 512], ps, bias_tiles[nm])

                  # v -> [s, ch] with ones columns: [v_h0|1] ; [v_h1|1]
                  vv = vva.tile([128, sbk, 256], F16, tag="vv")
                  nc.vector.memset(vv[:, :, 64:128], 1.0)
                  nc.vector.memset(vv[:, :, 192:256], 1.0)
                  for i in range(sbk):
                      vps = ps_mm.tile([128, 128], F16, tag="mm")
                      nc.tensor.transpose(
                          vps, vT[:, 128 * i:128 * i + 128], identity)
                      nc.vector.tensor_copy(vv[:, i, 0:64], vps[:, 0:64])
                      nc.vector.tensor_copy(vv[:, i, 128:192], vps[:, 64:128])

                  for j in range(tch):
                      yps = []
                      for h in range(2):
                          yp = ps_y.tile([128, 512], F32, tag=f"yp{h}")
                          yps.append(yp)
                      nsb = 4 * j + 4
                      for i in range(nsb):
                          toff = max(0, 128 * i - 512 * j)
                          w = 512 - toff
                          for h in range(2):
                              d0 = 64 * h
                              sp = ps_s.tile([128, 512], F32, tag=f"sp{h}")
                              nc.tensor.matmul(
                                  sp[:, :w],
                                  kT[d0:d0 + 64, 128 * i:128 * i + 128],
                                  qT[d0:d0 + 64, 512 * j + toff:512 * j + 512],
                                  start=True, stop=True,
                              )
                              if 128 * i >= 512 * j:
                                  nc.vector.tensor_add(
                                      sp[:, 0:128], sp[:, 0:128], cmask)
                              pt = ptile.tile([128, 512], F16, tag=f"p{h}")
                              nc.scalar.activation(
                                  pt[:, :w], sp[:, :w], EXP, scale=float(SCALE))
                              nc.tensor.matmul(
                                  yps[h][:, toff:512],
                                  vv[:, i, 128 * h:128 * h + 128],
                                  pt[:, :w],
                                  start=(i == 0), stop=(i == nsb - 1),
                                  skip_group_check=True,
                              )
                      g512 = (bi * t + 512 * j) // 512
                      dest = g512 % ncores
                      epoch = g512 // ncores
                      for h in range(2):
                          zr = ynorm.tile([64, 512], F32, tag="zr")
                          nc.vector.reciprocal(zr, yps[h][64:128, :])
                          yt = ynorm.tile([64, 512], F16, tag="yt")
                          nc.vector.tensor_mul(yt, yps[h][0:64, :], zr)
                          nc.sync.dma_start(
                              out=a2a_ins[epoch][dest, 64 * h:64 * h + 64,
                                                 0:512],
                              in_=yt)
                  # issue the epoch's collective right after its last batch
                  if bi % (b // 2) == (b // 2) - 1:
                      ep = bi // (b // 2)
                      nc.gpsimd.collective_compute(
                          "AllToAll", mybir.AluOpType.bypass,
                          replica_groups=[list(range(ncores))],
                          ins=[a2a_ins[ep].opt()], outs=[a2a_outs[ep].opt()],
                      )

              # --- output projection with full Wp on local token slice ---
              for jt in range(n_out_ch):
                  ygs = []
                  for ck in range(nk):
                      yg = ygpool.tile([128, 512], F16, tag="yg")
                      nc.sync.dma_start(
                          out=yg, in_=a2a_outs[jt][ck, :, 0:512])
                      ygs.append(yg)
                  for ot in range(nk):
                      wps = []
                      for ck in range(nk):
                          wpt = wppool.tile([128, 128], F16, tag="wp")
                          nc.sync.dma_start(
                              out=wpt,
                              in_=wpT[128 * ck:128 * ck + 128,
                                      128 * ot:128 * ot + 128])
                          wps.append(wpt)
                      ops = ps_mm.tile([128, 512], F32, tag="mm")
                      for ck in range(nk):
                          nc.tensor.matmul(
                              ops, wps[ck], ygs[ck],
                              start=(ck == 0), stop=(ck == nk - 1))
                      osb = outsb.tile([128, 512], F32, tag="osb")
                      nc.vector.tensor_copy(osb, ops)
                      nc.sync.dma_start(
                          out=outT[128 * ot:128 * ot + 128,
                                   512 * jt:512 * jt + 512],
                          in_=osb)
    nc.compile()
    return nc


_PROGRAM_CACHE = {}


def _get_program(key=(B, T, C, NCORES)):
    if key not in _PROGRAM_CACHE:
        _PROGRAM_CACHE[key] = build_program(*key)
    return _PROGRAM_CACHE[key]


def make_in_maps(x, Wq, bq, Wk, bk, Wv, bv, Wp, ncores=NCORES):
    bt = x.shape[0] * x.shape[1]
    c = x.shape[2]
    xT = np.ascontiguousarray(x.reshape(bt, c).T.astype(np.float16))
    wpT = np.ascontiguousarray(Wp.T.astype(np.float16))
    ident = np.eye(128, dtype=np.float16)
    in_maps = []
    for core in range(ncores):
        s = slice(128 * core, 128 * core + 128)
        in_maps.append({
            "xT": xT,
            "wqT": np.ascontiguousarray(Wq[s, :].T.astype(np.float16)),
            "wkT": np.ascontiguousarray(Wk[s, :].T.astype(np.float16)),
            "wvT": np.ascontiguousarray(Wv[s, :].T.astype(np.float16)),
            "bq": np.ascontiguousarray(bq[s].reshape(128, 1), dtype=np.float32),
            "bk": np.ascontiguousarray(bk[s].reshape(128, 1), dtype=np.float32),
            "bv": np.ascontiguousarray(bv[s].reshape(128, 1), dtype=np.float32),
            "wpT": wpT,
            "ident": ident,
        })
    return in_maps


def assemble_output(results, b=B, t=T, c=C, bp=None):
    bt = b * t
    n = len(results)
    out = np.empty((bt, c), np.float32)
    half = bt // 2
    for core, res in enumerate(results):
        oT = res["outT"]
        out[512 * core:512 * core + 512, :] = oT[:, 0:512].T
        out[half + 512 * core:half + 512 * core + 512, :] = oT[:, 512:1024].T
    out = out.reshape(b, t, c)
    if bp is not None:
        out = out + bp
    return out


def kernel(x, Wk, bk, Wq, bq, Wv, bv, Wp, bp, _trace=False):
    from concourse.bass_utils import run_bass_kernel_spmd

    x = np.asarray(x, np.float32)
    nc = _get_program()
    in_maps = make_in_maps(x, np.asarray(Wq), np.asarray(bq), np.asarray(Wk),
                           np.asarray(bk), np.asarray(Wv), np.asarray(bv),
                           np.asarray(Wp))
    res = run_bass_kernel_spmd(nc, in_maps, list(range(NCORES)), trace=_trace)
    out = assemble_output(res.results, bp=np.asarray(bp, np.float32))
    if _trace:
        return out, res
    return out

